# revision 41
# baseline (speedup 1.0000x reference)
"""Trainium2 Bass kernel v2 for nn_MicroExpert (sparse_attention).

Reference model (B=2, T=2048, D=512, H=8, HD=64):
  v_in = conv1d(x, k=3, pad=1); MHA(q=x, k=x, v=v_in) with banded mask
  |i-j| <= 256; h = LN(x + attn); out = LN(h + FFN(h)).

Sharding: data-parallel over (batch, 512-token chunk) -> 8 independent
cores, no collectives.  Each core recomputes the K/V halo (+-256 tokens,
zero-padded at sequence edges; pad keys are neutralized exactly via a
denominator correction `padcnt`).

v3 changes vs v2:
  - DMA consolidation: 13 loads -> 6 packed images (xt|wk|mask, wq,
    xt8|wu8 fp8, wo|w1|w2, row consts, biases), so the ~650ns-per-issue
    serial dma_start cost stops delaying the first Kproj matmul.
  - PE warm-up burst + early exp ACT-table preload during the DMA wait
    (HAM at K=8/8 when real work starts; no table load on the A1 path).
  - Vproj in fp8e4 DoubleRow: dc-pairs of K=128 slabs per MM (6 MMs/kv
    tile instead of 12); wu pre-scaled x64 on the host, undone at the
    psum drain (fp8 subnormal avoidance).
  - LN rstd via one Abs_reciprocal_sqrt activation (scale=1/D^2,
    bias=eps) replacing the sqrt -> reciprocal -> scale chain.
  - K/Q psum drains alternate DVE/Scalar; LN2 broadcast cast to bf16
    SBUF so the final normalize runs at DVE 2x; output streamed in 4
    512-col DMA pieces as each lands.

v2 (retained): transposed-layout LN via ones-column/ones-row matmuls,
[128,1280] stacked score tiles exp'd in one ACTIVATE with row-tiled
score matmuls, softmax denominator via a ones-row in v accumulated
during ctx, banded-mask neutralization via the padcnt correction.
"""

import sys

import numpy as np

sys.path.insert(0, "/opt/trn_rl_repo")

import concourse.bass as bass
import concourse.mybir as mybir
import concourse.tile as tile
from concourse import bacc
from concourse.bass_utils import run_bass_kernel_spmd

BF16 = mybir.dt.bfloat16
F32 = mybir.dt.float32
FP8 = mybir.dt.float8e4
AF = mybir.ActivationFunctionType
OP = mybir.AluOpType
PM = mybir.MatmulPerfMode
XE8 = 1040       # fp8 x chunk stride (XE padded so k-pair stride %16 == 0)
US = 64.0        # fp8 wu pre-scale (undone at the vproj psum drain)

B, T, D, H, HD = 2, 2048, 512, 8, 64
S = 512          # tokens per core
KV = 1024        # extended kv tokens per core (S + 2*256)
XE = 1026        # x_ext width (KV + 2 for conv halo)
F = 1024         # FFN hidden
EPS = 1e-5
N_CORES = 8

_cached = {}


def _build_program():
    nc = bacc.Bacc("TRN2", target_bir_lowering=False, debug=False)

    def din(name, shape, dt):
        return nc.dram_tensor(name, shape, dt, kind="ExternalInput").ap()

    # all inputs are pre-layouted [128, N] SBUF images (host does the
    # packing).  Tensors are consolidated into a few wide images so the
    # serial ~650ns-per-dma_start issue cost doesn't delay compute:
    #   xtwkm = xt | wk | mask01        (needed first: Kproj + A1 mask)
    #   quw   = wq | wu                 (Qproj + Vproj)
    #   oww   = wo | w1 | w2            (out_proj + FFN)
    xtwk_d = din("xtwk", [128, 4 * XE + 2048], BF16)
    v8_d = din("v8", [128, 4 * XE8 + 6144], FP8)      # xt8 | wu8 (fp8 vproj)
    wqm_d = din("wqm", [128, 2048 + 256], BF16)       # wq | mask
    oww_d = din("oww", [128, 2048 + 4096 + 4096], BF16)
    rows_d = din("rows2", [1, 1536], BF16)            # nw1s | padcntb
    bias_d = din("bias12", [128, 12], F32)            # b1 | b2

    out_d = nc.dram_tensor("out", [128, 2048], BF16, kind="ExternalOutput").ap()

    with tile.TileContext(nc) as tc:
        from contextlib import ExitStack

        with ExitStack() as ctx:
            const = ctx.enter_context(tc.tile_pool(name="const", bufs=1))

            # ---- SBUF residents -----------------------------------------
            xtwk_sb = const.tile([128, 4 * XE + 2048], BF16)
            xt_sb = xtwk_sb[:, 0: 4 * XE]
            wk_sb = xtwk_sb[:, 4 * XE: 4 * XE + 2048]
            wqm_sb = const.tile([128, 2048 + 256], BF16)
            wq_sb = wqm_sb[:, 0:2048]
            mask_sb = wqm_sb[:, 2048:2304]
            v8_sb = const.tile([128, 4 * XE8 + 6144], FP8)
            xt8_sb = v8_sb[:, 0: 4 * XE8]
            wu8_sb = v8_sb[:, 4 * XE8: 4 * XE8 + 6144]
            oww_sb = const.tile([128, 2048 + 4096 + 4096], BF16)
            wo_sb = oww_sb[:, 0:2048]
            w1_sb = oww_sb[:, 2048: 2048 + 4096]
            w2_sb = oww_sb[:, 6144: 6144 + 4096]
            rows2_sb = const.tile([1, 1536], BF16)
            nw1s_sb = rows2_sb[:, 0:1024]
            padcntb_sb = rows2_sb[:, 1024:1536]
            bias_sb = const.tile([128, 12], F32)
            b1_sb = bias_sb[:, 0:8]
            b2_sb = bias_sb[:, 8:12]
            negones_sb = const.tile([1, 64], BF16)
            ones_sb = const.tile([128, 128], BF16)
            warm_sb = const.tile([128, 512], BF16)
            eps_sb = const.tile([1, 1], F32)

            kt_sb = const.tile([128, 4 * KV], BF16)    # [oc-block][kv]
            q_sb = const.tile([128, 4 * S], BF16)      # [oc-block][tok]
            v_sb = const.tile([128, 8 * 520], BF16)    # [kvtile][(v_h|1) x 8]
            ctx_sb = const.tile([128, 4 * 512], BF16)  # [pair-block][q]
            r1t_sb = const.tile([128, 4 * 512], BF16)  # [dc-block][tok]
            hnt_sb = const.tile([128, 4 * 512], BF16)  # LN1 out [dc-block][tok]
            m1_sb = const.tile([128, 8 * 512], BF16)   # [fc-block][tok]
            r2t_sb = const.tile([128, 4 * 512], BF16)  # [dc-block][tok]
            out_sb = const.tile([128, 4 * 512], BF16)
            rows_sb = const.tile([1, 4 * 1024 + 32], F32)  # stat scratch rows
            rowsb_sb = const.tile([1, 2 * 1024 + 512], BF16)  # bcast-rhs rows
            dcol_sb = const.tile([128, 1], BF16)

            # ---- loads: 5 consolidated DMAs, first-needed first ---------
            # (dma_start instructions issue serially at ~650ns each on the
            # Sync HWDGE ring and transfers drain FIFO, so order = priority)
            nc.sync.dma_start(xtwk_sb[:], xtwk_d[:])
            nc.sync.dma_start(v8_sb[:], v8_d[:])
            nc.sync.dma_start(wqm_sb[:], wqm_d[:])
            nc.sync.dma_start(oww_sb[:], oww_d[:])
            nc.sync.dma_start(rows2_sb[:], rows_d[:])
            nc.sync.dma_start(bias_sb[:], bias_d[:])
            # ones/warm memsets go on the (idle) DVE so the PE warm-up
            # is not gated on gpsimd's slow preamble
            nc.vector.memset(ones_sb[:], 1.0)
            nc.vector.memset(warm_sb[:], 0.0)
            nc.gpsimd.memset(negones_sb[:], -1.0)
            nc.gpsimd.memset(eps_sb[:], float(EPS))
            nc.gpsimd.memset(dcol_sb[:], float(D))
            nc.gpsimd.memset(v_sb[:], 1.0)

            def hb(scratch_ps, dep_row, f32=True):
                # HAM-warming pokes measured neutral-to-negative; disabled
                return

            xt_v = xt_sb.rearrange("p (c w) -> p c w", c=4)

            # preload the exp ACT table set (~2.7us) while DMAs stream;
            # scalar is otherwise idle until the first softmax exp
            nc.scalar.activation(
                rows_sb[0:1, 0:16], ones_sb[0:1, 0:16], AF.Exp)

            # ---- PE warm-up: dummy matmuls during the DMA wait ----------
            # keeps the PE-HAM activity window busy so Kproj starts at
            # K=8/8 (2.4 GHz) instead of paying the ~3.4us cold ramp, and
            # eats the idle gap before the first real matmul
            with tc.tile_pool(name="warm", bufs=1, space="PSUM") as wpool:
                wt = wpool.tile([128, 512], F32, tag="w")
                for _ in range(16):
                    nc.tensor.matmul(
                        wt[:], ones_sb[:, 0:128], warm_sb[:],
                        start=True, stop=True)

            # ---- K/Q projections ---------------------------------------
            # psum drains alternate DVE/Scalar so neither engine's copy
            # chain gates the 2-buf psum ring
            with tc.tile_pool(name="pp", bufs=2, space="PSUM") as pp_pool:
                nd = 0
                for oc in range(4):
                    for half in range(2):
                        pp = pp_pool.tile([128, 512], F32, tag="pp")
                        for kc in range(4):
                            nc.tensor.matmul(
                                pp[:],
                                wk_sb[:, 128 * (4 * kc + oc):128 * (4 * kc + oc) + 128],
                                xt_sb[:, XE * kc + 1 + 512 * half: XE * kc + 1 + 512 * half + 512],
                                start=(kc == 0), stop=(kc == 3),
                            )
                        dst = kt_sb[:, KV * oc + 512 * half: KV * oc + 512 * half + 512]
                        if nd % 2 == 0:
                            nc.vector.tensor_copy(dst, pp[:])
                        else:
                            nc.scalar.copy(dst, pp[:])
                        nd += 1
                for oc in range(4):
                    pp = pp_pool.tile([128, 512], F32, tag="pp")
                    for kc in range(4):
                        nc.tensor.matmul(
                            pp[:],
                            wq_sb[:, 128 * (4 * kc + oc):128 * (4 * kc + oc) + 128],
                            xt_sb[:, XE * kc + 257: XE * kc + 257 + 512],
                            start=(kc == 0), stop=(kc == 3),
                        )
                    dst = q_sb[:, 512 * oc: 512 * oc + 512]
                    if nd % 2 == 0:
                        nc.vector.tensor_copy(dst, pp[:])
                    else:
                        nc.scalar.copy(dst, pp[:])
                    nd += 1

            # ---- A1: scores (+exp) interleaved with Vproj ---------------
            # chunk (p, qt): [128,1280] psum: head 2p scores cols 0:640
            # (5 kt tiles of 128), head 2p+1 at 640:1280.
            ex_tiles = {}

            def emit_scores(sc_pool, ex_pool, p, qt):
                sc = sc_pool.tile([128, 1280], F32, tag="sc")
                for r in range(5):
                    for par in range(2):
                        hp = 64 * par
                        nc.tensor.matmul(
                            sc[:, 640 * par + 128 * r: 640 * par + 128 * r + 128],
                            kt_sb[hp:hp + 64, KV * p + 128 * (qt + r): KV * p + 128 * (qt + r) + 128],
                            q_sb[hp:hp + 64, 512 * p + 128 * qt: 512 * p + 128 * qt + 128],
                            start=True, stop=True,
                        )
                ex = ex_pool.tile([128, 1280], BF16, tag="ex")
                nc.scalar.activation(
                    ex[:], sc[:], AF.Exp, scale=float(1.0 / np.sqrt(HD)),
                )
                # band mask on relative tiles 0 and 4 of each head
                exr = ex[:].rearrange("p (a b) -> p a b", a=10)
                mv = mask_sb[:].rearrange("p (n w) -> p n w", n=2)
                nc.vector.tensor_mul(exr[:, 0:5:4, :], exr[:, 0:5:4, :], mv)
                nc.vector.tensor_mul(exr[:, 5:10:4, :], exr[:, 5:10:4, :], mv)
                ex_tiles[(p, qt)] = ex

            x8v = xt8_sb.rearrange("p (c w) -> p c w", c=4)     # [128,4,1040]
            w8v = wu8_sb.rearrange("p (b w) -> p b w", b=12)    # [128,12,512]

            def emit_vproj(v_pool, tt):
                # fp8 DoubleRow: each matmul consumes a dc-PAIR of K=128
                # slabs (lhsT [128,2,128] x-chunks, rhs [128,2,512] wu
                # blocks) -> 6 MMs instead of 12 at ~1.13x per-MM cost.
                # wu is pre-scaled by US on the host; undone at the drain.
                pp = v_pool.tile([128, 512], F32, tag="vp")
                n = 0
                off = 128 * tt
                for tap in range(3):
                    for dcp in range(2):
                        nc.tensor.matmul(
                            pp[:],
                            x8v[:, 2 * dcp: 2 * dcp + 2, off + tap: off + tap + 128],
                            w8v[:, 4 * tap + 2 * dcp: 4 * tap + 2 * dcp + 2, :],
                            perf_mode=PM.DoubleRow,
                            start=(n == 0), stop=(n == 5),
                        )
                        n += 1
                vv = v_sb[:, 520 * tt: 520 * tt + 520].rearrange(
                    "p (h w) -> p h w", h=8
                )
                nc.vector.tensor_scalar_mul(
                    vv[:, :, 0:64], pp[:].rearrange("p (h w) -> p h w", h=8),
                    float(1.0 / US),
                )

            with ExitStack() as aouter:
                ex_pool = aouter.enter_context(tc.tile_pool(name="exsb", bufs=16))

                with ExitStack() as a1:
                    sc_pool = a1.enter_context(
                        tc.tile_pool(name="scps", bufs=2, space="PSUM"))
                    v_pool = a1.enter_context(
                        tc.tile_pool(name="vps", bufs=2, space="PSUM"))

                    chunks = [(p, qt) for p in range(4) for qt in range(4)]
                    emit_scores(sc_pool, ex_pool, *chunks[0])
                    emit_scores(sc_pool, ex_pool, *chunks[1])
                    ci = 2
                    for tt in range(8):
                        emit_vproj(v_pool, tt)
                        for _ in range(2):
                            if ci < 16:
                                emit_scores(sc_pool, ex_pool, *chunks[ci])
                                ci += 1

                # dummy rsqrt: pulls the rsqrt ACT_TABLE_LOAD off the LN1
                # critical path (loads while the PE runs ctx matmuls).
                # scale=0/bias=1 keeps the input in-range (ex can be 0) while
                # preserving the dependency on the last exp tile
                nc.scalar.activation(
                    rows_sb[0:1, 4096:4112], ex_tiles[(3, 3)][0:1, 0:16],
                    AF.Abs_reciprocal_sqrt, bias=1.0, scale=0.0)

                # ---- A2: ctx accumulation + per-pair normalize ----------
                with ExitStack() as a2:
                    cx_pool = a2.enter_context(
                        tc.tile_pool(name="cxps", bufs=3, space="PSUM"))
                    bc_pool = a2.enter_context(
                        tc.tile_pool(name="bcps", bufs=2, space="PSUM"))
                    dn_pool = a2.enter_context(tc.tile_pool(name="dnsb", bufs=3))

                    def norm_front(p, cps):
                        # den rows (psum f32 -> sbuf bf16 casts, on Scalar)
                        den = dn_pool.tile([1, 1024], BF16, tag="den")
                        nc.scalar.copy(den[0:1, 0:512], cps[0][64:65, :])
                        nc.scalar.copy(den[0:1, 512:1024], cps[1][64:65, :])
                        return den

                    def norm_back(p, cps, den):
                        # bcast (den - padcnt) via two accumulating ones-row
                        # matmuls per half, then wide [128,512] DVE ops
                        bc = bc_pool.tile([128, 512], F32, tag="bc")
                        for par in range(2):
                            nc.tensor.matmul(
                                bc[64 * par:64 * par + 64, :],
                                ones_sb[0:1, 0:64],
                                den[0:1, 512 * par: 512 * par + 512],
                                start=True, stop=False,
                                skip_group_check=True,
                                tile_position=(0, 64 * par),
                            )
                            nc.tensor.matmul(
                                bc[64 * par:64 * par + 64, :],
                                negones_sb[0:1, 0:64],
                                padcntb_sb[0:1, :],
                                start=False, stop=True,
                                skip_group_check=True,
                                tile_position=(0, 64 * par),
                            )
                        recf = dn_pool.tile([128, 512], F32, tag="recf")
                        nc.vector.reciprocal_approx_fast(recf[:], bc[:])
                        hb(bc, recf[0:1, 0:64])
                        for par in range(2):
                            nc.vector.tensor_mul(
                                ctx_sb[64 * par:64 * par + 64, 512 * p: 512 * p + 512],
                                cps[par][0:64, :],
                                recf[64 * par:64 * par + 64, :],
                            )
                        hb(bc, ctx_sb[0:1, 512 * p: 512 * p + 64], f32=False)

                    pending = None
                    for p in range(4):
                        cxA = cx_pool.tile([65, 512], F32, tag="cxA")
                        cxB = cx_pool.tile([65, 512], F32, tag="cxB")
                        cps = (cxA, cxB)
                        for qt in range(4):
                            ex = ex_tiles[(p, qt)]
                            for r in range(5):
                                k = qt + r
                                for par in range(2):
                                    nc.tensor.matmul(
                                        cps[par][0:65, 128 * qt: 128 * qt + 128],
                                        v_sb[:, 520 * k + 65 * (2 * p + par): 520 * k + 65 * (2 * p + par) + 65],
                                        ex[:, 640 * par + 128 * r: 640 * par + 128 * r + 128],
                                        start=(qt == 0 and r == 0),
                                        stop=(qt == 3 and r == 4),
                                        skip_group_check=True,
                                    )
                        rdenb = norm_front(p, cps)
                        if pending is not None:
                            norm_back(*pending)
                        pending = (p, cps, rdenb)
                    norm_back(*pending)

                    # out_proj inside the A2 scope; pc-major in two 2-bank
                    # passes so the first accumulation MMs start as soon as
                    # each norm_back delivers its ctx chunk instead of the
                    # whole oc chain waiting on the last chunk
                    for g in range(2):
                        atps = [
                            bc_pool.tile([128, 512], F32, tag="bc",
                                         name=f"atp{2 * g + j}")
                            for j in range(2)
                        ]
                        for pc in range(4):
                            for j in range(2):
                                oc = 2 * g + j
                                nc.tensor.matmul(
                                    atps[j][:],
                                    wo_sb[:, 128 * (4 * pc + oc): 128 * (4 * pc + oc) + 128],
                                    ctx_sb[:, 512 * pc: 512 * pc + 512],
                                    start=(pc == 0), stop=(pc == 3),
                                )
                        for j in range(2):
                            oc = 2 * g + j
                            nc.vector.tensor_add(
                                r1t_sb[:, 512 * oc: 512 * oc + 512],
                                atps[j][:], xt_v[:, oc, 257:769],
                            )

            # ---- out_proj + residual + LN1 ------------------------------
            def ln_rows(stats_ps, base, bc):
                # rstd = Rsqrt((D*ssq - s^2)/D^2 + eps) in ONE activation;
                # rowsb gets (mean | rstd) bf16
                s_ps = stats_ps[0:1, 0:512]
                ssq_ps = stats_ps[0:1, 512:1024]
                m2 = rows_sb[0:1, 2048 + base: 2048 + base + 512]
                nc.scalar.activation(m2, s_ps, AF.Square)          # s^2
                t = rows_sb[0:1, 2048 + base + 512: 2048 + base + 1024]
                nc.scalar.mul(rowsb_sb[0:1, base: base + 512], s_ps, 1.0 / D)
                nc.vector.tensor_sub(t, ssq_ps, m2)                # D^2 * var
                nc.scalar.activation(rowsb_sb[0:1, base + 512: base + 1024],
                                     t, AF.Abs_reciprocal_sqrt, bias=eps_sb[0:1, 0:1],
                                     scale=float(1.0 / (D * D)))

            def ln_bcast(bc, base):
                for half in range(2):
                    nc.tensor.matmul(
                        bc[:, 512 * half: 512 * half + 512],
                        ones_sb[0:1, 0:128],
                        rowsb_sb[0:1, base + 512 * half: base + 512 * half + 512],
                        start=True, stop=True, skip_group_check=True,
                    )

            # ---- out_proj + LN1 + FFN1 (FFN1 GEMMs run on the raw
            # residual r1t; the LN mean folds in as a rank-1 PE
            # accumulation and rstd applies columnwise at drain time, so
            # the whole LN1 stats chain hides under the GEMMs) ----------
            with ExitStack() as o1:
                # stats and bc1 share one 2-bank slot sequentially (bc1 is
                # only written after the chain has consumed stats), so
                # f_pool below can hold 6 GEMM buffers
                st_pool = o1.enter_context(
                    tc.tile_pool(name="stps", bufs=1, space="PSUM"))
                sq_pool = o1.enter_context(tc.tile_pool(name="sqsb", bufs=2))
                tm_pool = o1.enter_context(tc.tile_pool(name="tmsb", bufs=4))

                stats = st_pool.tile([1, 1024], F32, tag="stlb")
                for dc in range(4):
                    sl = slice(512 * dc, 512 * dc + 512)
                    sq = sq_pool.tile([128, 512], BF16, tag="sq")
                    nc.scalar.activation(sq[:], r1t_sb[:, sl], AF.Square)
                    nc.tensor.matmul(
                        stats[0:1, 0:512], ones_sb[:, 0:1], r1t_sb[:, sl],
                        start=(dc == 0), stop=(dc == 3), skip_group_check=True,
                    )
                    nc.tensor.matmul(
                        stats[0:1, 512:1024], dcol_sb[:, 0:1], sq[:],
                        start=(dc == 0), stop=(dc == 3), skip_group_check=True,
                    )
                # atps banks released here -> f_pool can take 6
                f_pool = o1.enter_context(
                    tc.tile_pool(name="fps", bufs=6, space="PSUM"))

                phi = rowsb_sb[0:1, 2048: 2048 + 512]

                def ffn1_g(fc):
                    g = f_pool.tile([128, 512], F32, tag="f")
                    for dc in range(4):
                        nc.tensor.matmul(
                            g[:],
                            w1_sb[:, 128 * (8 * dc + fc): 128 * (8 * dc + fc) + 128],
                            r1t_sb[:, 512 * dc: 512 * dc + 512],
                            start=(dc == 0), stop=False, skip_group_check=True,
                        )
                    return g

                def ffn1_fin(fc, g):
                    # G += (-sum_d w1) (x) (mu*rstd); m1 = Relu(G*rstd + b1)
                    nc.tensor.matmul(
                        g[:], nw1s_sb[0:1, 128 * fc: 128 * fc + 128], phi,
                        start=False, stop=True, skip_group_check=True,
                    )
                    t1 = tm_pool.tile([128, 512], BF16, tag="t1")
                    nc.vector.tensor_mul(t1[:], g[:], bc1s[:, 512:1024])
                    nc.scalar.activation(
                        m1_sb[:, 512 * fc: 512 * fc + 512], t1[:],
                        AF.Relu, bias=b1_sb[:, fc:fc + 1],
                    )

                gq = []
                for fc in range(5):
                    gq.append((fc, ffn1_g(fc)))
                ln_rows(stats, 0, stats)
                nc.vector.tensor_mul(
                    phi, rowsb_sb[0:1, 0:512], rowsb_sb[0:1, 512:1024])
                bc1 = st_pool.tile([128, 1024], F32, tag="stlb", name="bc1t")
                ln_bcast(bc1, 0)
                bc1s = tm_pool.tile([128, 1024], BF16, tag="bcs")
                nc.vector.tensor_copy(bc1s[:], bc1[:])
                for fc in range(5, 8):
                    gq.append((fc, ffn1_g(fc)))
                    ffn1_fin(*gq.pop(0))
                while gq:
                    ffn1_fin(*gq.pop(0))
                # hnt (normalized h) still needed for the second residual;
                # b2 (per-feature) is folded in here so FFN2 can skip its
                # bias activation and add the GEMM psum to hnt directly
                tmps = []
                for dc in range(4):
                    sl = slice(512 * dc, 512 * dc + 512)
                    tmp = tm_pool.tile([128, 512], BF16, tag="tm")
                    nc.vector.tensor_sub(tmp[:], r1t_sb[:, sl], bc1s[:, 0:512])
                    tmps.append(tmp)
                for dc in range(4):
                    sl = slice(512 * dc, 512 * dc + 512)
                    nc.vector.tensor_mul(
                        tmps[dc][:], tmps[dc][:], bc1s[:, 512:1024])
                for dc in range(4):
                    sl = slice(512 * dc, 512 * dc + 512)
                    nc.vector.tensor_scalar_add(
                        hnt_sb[:, sl], tmps[dc][:], b2_sb[:, dc:dc + 1])

            # ---- FFN2 + residual + LN2 + store, pipelined over token
            # halves: the left half's LN2 chain + normalize + store run
            # while the PE computes the right half's GEMMs ---------------
            with ExitStack() as f1:
                # per-half stats and bc broadcast share a 2-bank slot
                # (bc is only written after the chain consumed the stats),
                # so f_pool gets 4 GEMM buffers instead of 2
                f_pool = f1.enter_context(
                    tc.tile_pool(name="fps2", bufs=4, space="PSUM"))
                lb_pool = f1.enter_context(
                    tc.tile_pool(name="lbps2", bufs=1, space="PSUM"))
                sq_pool = f1.enter_context(tc.tile_pool(name="sqsb2", bufs=2))
                tm_pool = f1.enter_context(tc.tile_pool(name="tmsb2", bufs=3))

                statsh = [lb_pool.tile([1, 1024], F32, tag=f"sl{h}",
                                       name=f"st2h{h}")
                          for h in range(2)]
                bch = {}

                def st2_mms(h, oc, sq):
                    sl = slice(512 * oc + 256 * h, 512 * oc + 256 * h + 256)
                    nc.tensor.matmul(
                        statsh[h][0:1, 0:256],
                        ones_sb[:, 0:1], r2t_sb[:, sl],
                        start=(oc == 0), stop=(oc == 3), skip_group_check=True,
                    )
                    nc.tensor.matmul(
                        statsh[h][0:1, 512:768],
                        dcol_sb[:, 0:1], sq[:],
                        start=(oc == 0), stop=(oc == 3), skip_group_check=True,
                    )

                def ln2_rows(h):
                    # rstd = Rsqrt((D*ssq - s^2)/D^2 + eps), one activation
                    base = 3072 + 512 * h
                    s_ps = statsh[h][0:1, 0:256]
                    ssq_ps = statsh[h][0:1, 512:768]
                    ob = 1024 + 512 * h
                    nc.scalar.mul(rowsb_sb[0:1, ob: ob + 256], s_ps, 1.0 / D)
                    m2 = rows_sb[0:1, base: base + 256]
                    nc.scalar.activation(m2, s_ps, AF.Square)
                    t = rows_sb[0:1, base + 256: base + 512]
                    nc.vector.tensor_sub(t, ssq_ps, m2)
                    nc.scalar.activation(rowsb_sb[0:1, ob + 256: ob + 512],
                                         t, AF.Abs_reciprocal_sqrt, bias=eps_sb[0:1, 0:1],
                                         scale=float(1.0 / (D * D)))

                def ln2_bc(h):
                    ob = 1024 + 512 * h
                    bch[h] = lb_pool.tile([128, 512], F32, tag=f"sl{h}",
                                          name=f"bch{h}")
                    for half2 in range(2):
                        nc.tensor.matmul(
                            bch[h][:, 256 * half2: 256 * half2 + 256],
                            ones_sb[0:1, 0:128],
                            rowsb_sb[0:1, ob + 256 * half2: ob + 256 * half2 + 256],
                            start=True, stop=True, skip_group_check=True,
                        )

                def ln2_out(h):
                    # cast the psum broadcast rows to bf16 SBUF once so the
                    # four normalize TTs run at DVE 2x instead of 1x, and
                    # stream each 512-col piece to HBM as soon as it lands
                    bchs = tm_pool.tile([128, 512], BF16, tag="bcsh")
                    nc.vector.tensor_copy(bchs[:], bch[h][:])
                    r2v = r2t_sb[:].rearrange("p (dc w) -> p dc w", dc=8)
                    ov = out_sb[:].rearrange("p (g w) -> p g w", g=8)
                    muv = bchs[:, 0:256].rearrange(
                        "p (g w) -> p g w", g=1).broadcast_to([128, 2, 256])
                    rsv = bchs[:, 256:512].rearrange(
                        "p (g w) -> p g w", g=1).broadcast_to([128, 2, 256])
                    # the two output DMAs issue on separate queues so the
                    # final drain is not serialized on the Sync ring
                    tvs = []
                    for dp in range(2):
                        # dc pair (2*dp, 2*dp+1), half h columns
                        tmp = tm_pool.tile([128, 512], BF16, tag="tmo")
                        tv = tmp[:].rearrange("p (g w) -> p g w", g=2)
                        nc.vector.tensor_sub(
                            tv, r2v[:, 4 * dp + h:4 * dp + h + 3:2, :], muv)
                        tvs.append(tv)
                    for dp in range(2):
                        nc.vector.tensor_mul(
                            ov[:, 4 * h + 2 * dp: 4 * h + 2 * dp + 2, :],
                            tvs[dp], rsv)
                        lo = 256 * (4 * h + 2 * dp)
                        dq = nc.scalar if dp == 0 else nc.sync
                        dq.dma_start(
                            out_d[:, lo: lo + 512], out_sb[:, lo: lo + 512])

                def gemms(h, mid=None):
                    pend = None
                    for oc in range(4):
                        sl = slice(512 * oc + 256 * h, 512 * oc + 256 * h + 256)
                        fps = f_pool.tile([128, 512], F32, tag="f")
                        for fc in range(8):
                            nc.tensor.matmul(
                                fps[:, 0:256],
                                w2_sb[:, 128 * (4 * fc + oc): 128 * (4 * fc + oc) + 128],
                                m1_sb[:, 512 * fc + 256 * h: 512 * fc + 256 * h + 256],
                                start=(fc == 0), stop=(fc == 7),
                            )
                        if pend is not None:
                            st2_mms(h, *pend)
                        if mid is not None and oc in mid:
                            mid[oc]()
                        nc.vector.tensor_add(
                            r2t_sb[:, sl], fps[:, 0:256], hnt_sb[:, sl])
                        sq = sq_pool.tile([128, 256], BF16, tag="sq2")
                        nc.scalar.activation(sq[:], r2t_sb[:, sl], AF.Square)
                        pend = (oc, sq)
                    st2_mms(h, *pend)

                gemms(0)
                ln2_rows(0)

                def mid0a():
                    ln2_bc(0)

                def mid0b():
                    ln2_out(0)

                gemms(1, mid={1: mid0a, 3: mid0b})
                ln2_rows(1)
                ln2_bc(1)
                ln2_out(1)

    nc.compile()
    return nc


def _prep_host(inputs):
    x = np.asarray(inputs["x"], np.float32)
    conv_w = np.asarray(inputs["conv_w"], np.float32)
    conv_b = np.asarray(inputs["conv_b"], np.float32)
    in_w = np.asarray(inputs["in_proj_w"], np.float32)
    in_b = np.asarray(inputs["in_proj_b"], np.float32)
    out_w = np.asarray(inputs["out_proj_w"], np.float32)
    out_b = np.asarray(inputs["out_proj_b"], np.float32)
    w1 = np.asarray(inputs["w1"], np.float32)
    b1 = np.asarray(inputs["b1"], np.float32)
    w2 = np.asarray(inputs["w2"], np.float32)
    b2 = np.asarray(inputs["b2"], np.float32)
    g1 = np.asarray(inputs["ln1_g"], np.float32)
    bb1 = np.asarray(inputs["ln1_b"], np.float32)
    g2 = np.asarray(inputs["ln2_g"], np.float32)
    bb2 = np.asarray(inputs["ln2_b"], np.float32)

    for nm, v in (("conv_b", conv_b), ("in_proj_b", in_b), ("out_proj_b", out_b)):
        if np.any(v != 0):
            raise NotImplementedError(f"nonzero {nm} unsupported")
    if np.any(g1 != 1) or np.any(bb1 != 0) or np.any(g2 != 1) or np.any(bb2 != 0):
        raise NotImplementedError("nontrivial layernorm affine unsupported")

    Wq, Wk, Wv = in_w[:D], in_w[D:2 * D], in_w[2 * D:]
    U = [(Wv @ conv_w[:, :, d]) for d in range(3)]  # v[t] = sum U_d @ x[t+d-1]

    def img(stack):  # [n, 128, w] slices -> [128, n*w] SBUF image
        a = np.asarray(stack, np.float32)
        return np.ascontiguousarray(a.transpose(1, 0, 2).reshape(128, -1))

    def slc16(W):  # W used as out = W @ x  -> lhsT slices of W.T, oc-major
        WT = np.ascontiguousarray(W.T)
        return img([
            WT[128 * kc:128 * kc + 128, 128 * oc:128 * oc + 128]
            for kc in range(4) for oc in range(4)
        ])

    wk_a = slc16(Wk)
    wq_a = slc16(Wq)
    wo_a = slc16(out_w)
    wu_a = img([
        np.ascontiguousarray(U[tap].T)[128 * dc:128 * dc + 128, :]
        for tap in range(3) for dc in range(4)
    ])
    w1_a = img([
        np.ascontiguousarray(w1.T)[128 * dc:128 * dc + 128, 128 * fc:128 * fc + 128]
        for dc in range(4) for fc in range(8)
    ])
    w2_a = img([
        np.ascontiguousarray(w2.T)[128 * fc:128 * fc + 128, 128 * oc:128 * oc + 128]
        for fc in range(8) for oc in range(4)
    ])
    b1_a = np.ascontiguousarray(b1.reshape(8, 128).T)
    b2_a = np.ascontiguousarray(b2.reshape(4, 128).T)

    r = np.arange(128)
    m_lo = (r[:, None] >= r[None, :]).astype(np.float32)   # block 0: keep k>=q
    mask01 = np.concatenate([m_lo, m_lo.T], axis=1)

    import ml_dtypes

    def bf(a):
        return np.asarray(a, dtype=ml_dtypes.bfloat16)

    def f8(a):
        return np.asarray(np.clip(a, -224.0, 224.0),
                          dtype=ml_dtypes.float8_e4m3)

    nw1s_a = -w1.sum(axis=1).reshape(1, 1024)
    wu8_a = f8(wu_a * 64.0)    # US pre-scale keeps wu out of fp8 subnormals
    common = {
        "wqm": bf(np.concatenate([wq_a, mask01], axis=1)),
        "oww": bf(np.concatenate([wo_a, w1_a, w2_a], axis=1)),
        "bias12": np.concatenate([b1_a, b2_a], axis=1).astype(np.float32),
    }

    in_maps = []
    for c in range(N_CORES):
        b, j = divmod(c, 4)
        s = 512 * j
        xe = np.zeros((XE, D), np.float32)
        lo, hi = max(0, s - 257), min(T, s + 769)
        xe[lo - (s - 257): hi - (s - 257)] = x[b, lo:hi]
        xt = xe.T.reshape(4, 128, XE).transpose(1, 0, 2).reshape(128, 4 * XE)
        xt = np.ascontiguousarray(xt)

        # padcnt[qt, r]: in-band-kept pad keys
        key = (s - 256 + 128 * np.arange(4)[:, None, None]
               + np.arange(640)[None, None, :])          # [qt,1,640]
        pad = (key < 0) | (key >= T)
        cc, rr = np.arange(640)[None, None, :], r[None, :, None]
        kept = ((cc >= 128) & (cc < 512)) | ((cc < 128) & (cc >= rr)) \
            | ((cc >= 512) & (cc - 512 <= rr))
        pc = (pad & kept).sum(axis=2).astype(np.float32)  # [4, 128]
        padcntb = pc.reshape(1, 512)

        # fp8 x image: same [128, 4, XE] layout padded to XE8 per chunk so
        # the DoubleRow k-pair stride (XE8 bytes) is 16B-aligned
        xt8 = np.zeros((128, 4, 1040), np.float32)
        xt8[:, :, 0:XE] = xt.reshape(128, 4, XE)
        xt8 = xt8.reshape(128, 4 * 1040)

        m = dict(common)
        m["xtwk"] = bf(np.concatenate([xt, wk_a], axis=1))
        m["v8"] = np.concatenate([f8(xt8), wu8_a], axis=1)
        m["rows2"] = bf(np.concatenate([nw1s_a, padcntb], axis=1))
        in_maps.append(m)
    return in_maps


def kernel(**inputs) -> np.ndarray:
    if "nc" not in _cached:
        _cached["nc"] = _build_program()
    nc = _cached["nc"]
    in_maps = _prep_host(inputs)
    res = run_bass_kernel_spmd(nc, in_maps, core_ids=list(range(N_CORES)))
    out = np.empty((B, T, D), np.float32)
    for c in range(N_CORES):
        b, j = divmod(c, 4)
        o = res.results[c]["out"].astype(np.float32).reshape(128, 2, 4, 256)
        out[b, 512 * j: 512 * j + 512] = \
            o.transpose(1, 3, 2, 0).reshape(512, 512)
    return out

